# revision 1
# baseline (speedup 1.0000x reference)
"""DeepSeek-V2-style MoE kernel for 8 Trainium2 NeuronCores.

Strategy (expert-parallel, dense):
- 16 experts, 8 cores -> 2 experts per core. Each core computes its two
  experts' SwiGLU MLPs densely over all 1024 tokens (bf16 matmuls, fp32
  accumulate), weighted by on-device routing weights.
- The shared expert is sharded over its intermediate dim (256 of 2048 per
  core) across all tokens; its per-core partial seeds the routed combine,
  so one ReduceScatter(add) of the [T, H] partial (as two [T, 512] column
  halves) produces each core's final 128-token output shard directly.
- The gate (sigmoid + grouped top-k) runs on every core in fp32 (matmul
  included) so expert selection matches the fp32 reference exactly. The
  expert axis is permuted per core (group structure preserved) so each
  core's own experts sit at positions 0 and 1 -> identical SPMD program.
"""

import os
import sys

import numpy as np
import ml_dtypes

for _p in ("/opt/trn_rl_repo", os.path.expanduser("~/.axon_site/_ro/trn_rl_repo")):
    if os.path.isdir(_p) and _p not in sys.path:
        sys.path.append(_p)

import concourse.bass as bass
import concourse.mybir as mybir
import concourse.tile as tile
from concourse.bass_utils import run_bass_kernel_spmd

# problem sizes (fixed)
T, H, E, I, SI = 1024, 1024, 16, 704, 2048
P = 128
NCORES = 8
KT = H // P            # 8 contraction tiles over H
IT = 6                 # ceil(704/128) I tiles; last is 64 rows (wd zero-padded)
IPAD = IT * P          # 768
SIC = SI // NCORES     # 256: shared-expert intermediate slice per core
SICT = SIC // P        # 2
NB = 2                 # token blocks
BLK = T // NB          # 512
MSUB = BLK // P        # 4 token subtiles per block
BIG = 1.0e6
OFF = 10.0             # offset making all valid masked scores positive

F32 = mybir.dt.float32
BF16 = mybir.dt.bfloat16
ALU = mybir.AluOpType
ACTF = mybir.ActivationFunctionType

_BUILD_CACHE = {}


def _split_sync_waits(nc):
    """This walrus build allows one sync wait per instruction; move extra
    waits onto same-engine pure-wait carriers placed immediately before."""
    n_split = 0
    for f in nc.m.functions:
        for bb in f.blocks:
            out = []
            for ins in bb.instructions:
                si = ins.sync_info
                if si is not None and si.on_wait and len(si.on_wait) > 1:
                    waits = list(si.on_wait)
                    head, tail = waits[:-1], waits[-1:]
                    for i, w in enumerate(head):
                        carrier = mybir.InstEventSemaphore(
                            name=f"{ins.name}-ws{i}",
                            engine=ins.engine,
                            ins=[],
                            outs=[],
                            sync_info=mybir.SyncInfo(on_wait=[w], on_update=[]),
                        )
                        nc.register_instruction(carrier, overwrite=True)
                        out.append(carrier)
                    ins.sync_info = mybir.SyncInfo(on_wait=tail,
                                                   on_update=si.on_update)
                    n_split += 1
                out.append(ins)
            bb.instructions[:] = out
    return nc


def _build(with_collective=True, routed_reps=1, shared_reps=1, coll_reps=1):
    nc = bass.Bass(num_devices=NCORES)

    # ---- parameters (per-core contents supplied host-side) ----
    ht = nc.declare_dram_parameter("ht", [H, T], BF16, isOutput=False)
    ht32 = nc.declare_dram_parameter("ht32", [8, P, KT, P], F32,
                                     isOutput=False)
    gw32 = nc.declare_dram_parameter("gw32", [H, E], F32, isOutput=False)
    bias_rep = nc.declare_dram_parameter("bias_rep", [P, P], F32, isOutput=False)
    wgu = [[nc.declare_dram_parameter(f"w{n}{e}", [H, I], BF16, isOutput=False)
            for n in ("g", "u")] for e in range(2)]
    wdp = [nc.declare_dram_parameter(f"wd{e}", [IPAD, H], BF16, isOutput=False)
           for e in range(2)]
    swg_my = nc.declare_dram_parameter("swg_my", [H, SIC], BF16, isOutput=False)
    swu_my = nc.declare_dram_parameter("swu_my", [H, SIC], BF16, isOutput=False)
    swd_my = nc.declare_dram_parameter("swd_my", [SIC, H], BF16, isOutput=False)
    out = nc.declare_dram_parameter("out", [P, H], F32, isOutput=True)

    with tile.TileContext(nc) as tc:
        with (
            tc.tile_pool(name="const", bufs=1) as const,
            tc.tile_pool(name="ht32s", bufs=1) as ht32s,
            tc.tile_pool(name="wpool", bufs=1) as wpool,
            tc.tile_pool(name="apool", bufs=2) as apool,
            tc.tile_pool(name="stmp", bufs=2) as stmp,
            tc.tile_pool(name="part", bufs=2) as part,
            tc.tile_pool(name="rpool", bufs=1) as rpool,
            tc.tile_pool(name="pgu", bufs=4, space="PSUM") as pgu,
            tc.tile_pool(name="py", bufs=4, space="PSUM") as py,
            tc.tile_pool(name="dram", bufs=1, space="DRAM") as dram,
        ):
            # ------------- gate operand loads (gate runs after shared G/U) --
            gw_sb = const.tile([P, KT, E], F32, name="gw_sb")
            nc.sync.dma_start(out=gw_sb[:],
                              in_=gw32.rearrange("(k p) e -> p k e", p=P))
            # ------------- constant + weight loads -------------
            ht_sb = const.tile([P, KT, T], BF16, name="ht_sb")
            for k in range(KT):
                nc.sync.dma_start(out=ht_sb[:, k, :],
                                  in_=ht[k * P:(k + 1) * P, :])
            bias_sb = const.tile([P, P], F32, name="bias_sb")
            nc.sync.dma_start(out=bias_sb[:], in_=bias_rep[:])

            swg_sb = wpool.tile([P, KT, SIC], BF16, name="swg_sb", tag="swg")
            swu_sb = wpool.tile([P, KT, SIC], BF16, name="swu_sb", tag="swu")
            nc.scalar.dma_start(out=swg_sb[:],
                                in_=swg_my.rearrange("(k p) c -> p k c", p=P))
            nc.scalar.dma_start(out=swu_sb[:],
                                in_=swu_my.rearrange("(k p) c -> p k c", p=P))
            swd_sb = wpool.tile([P, SICT, H], BF16, name="swd_sb", tag="swd")
            nc.scalar.dma_start(out=swd_sb[:],
                                in_=swd_my.rearrange("(i p) h -> p i h", p=P))

            scores = rpool.tile([P, P], F32, name="scores")
            hts_t = []
            _eng = [nc.sync, nc.scalar, nc.gpsimd]
            for tt in range(8):
                hts = ht32s.tile([P, KT, P], F32, name=f"hts{tt}",
                                 tag=f"hts{tt}")
                _eng[tt % 3].dma_start(out=hts[:], in_=ht32[tt])
                hts_t.append(hts)

            wg_sb, wu_sb, wd_sb = [], [], []
            for e in range(2):
                g_t = wpool.tile([P, KT, I], BF16, name=f"wg{e}_sb", tag=f"wg{e}")
                u_t = wpool.tile([P, KT, I], BF16, name=f"wu{e}_sb", tag=f"wu{e}")
                for k in range(KT):
                    nc.sync.dma_start(out=g_t[:, k, :],
                                      in_=wgu[e][0][k * P:(k + 1) * P, :])
                    nc.sync.dma_start(out=u_t[:, k, :],
                                      in_=wgu[e][1][k * P:(k + 1) * P, :])
                d_t = wpool.tile([P, IT, H], BF16, name=f"wd{e}_sb", tag=f"wd{e}")
                for i in range(IT):
                    nc.sync.dma_start(out=d_t[:, i, :],
                                      in_=wdp[e][i * P:(i + 1) * P, :])
                wg_sb.append(g_t)
                wu_sb.append(u_t)
                wd_sb.append(d_t)


            # ------------- shared expert (intermediate slice, all tokens) --
            As = const.tile([P, SICT, T], BF16, name="As_sh")
            ys = const.tile([P, NB * MSUB, 2, 512], BF16, name="ys")
            for rep_s in range(shared_reps):
                for si in range(SICT):
                    for b in range(NB):
                        tsl = slice(b * BLK, (b + 1) * BLK)
                        pGs = pgu.tile([P, 512], F32, name="pgs", tag="pgu")
                        pUs = pgu.tile([P, 512], F32, name="pus", tag="pgu")
                        for k in range(KT):
                            nc.tensor.matmul(
                                pGs[:, :], lhsT=swg_sb[:, k, si * P:(si + 1) * P],
                                rhs=ht_sb[:, k, tsl],
                                start=(k == 0), stop=(k == KT - 1))
                        for k in range(KT):
                            nc.tensor.matmul(
                                pUs[:, :], lhsT=swu_sb[:, k, si * P:(si + 1) * P],
                                rhs=ht_sb[:, k, tsl],
                                start=(k == 0), stop=(k == KT - 1))
                        sts = stmp.tile([P, BLK], F32, name="st", tag="st")
                        nc.scalar.activation(sts[:, :], pGs[:, :], ACTF.Silu)
                        nc.vector.tensor_tensor(As[:, si, tsl], sts[:, :],
                                                pUs[:, :], op=ALU.mult)
                if rep_s == 0:
                    for tt in range(8):
                        pg = pgu.tile([P, 512], F32, name="pgate", tag="pgu")
                        for k in range(KT):
                            nc.tensor.matmul(pg[:, :E],
                                             lhsT=hts_t[tt][:, k, :],
                                             rhs=gw_sb[:, k, :],
                                             start=(k == 0), stop=(k == KT - 1))
                        nc.scalar.activation(scores[:, tt * E:(tt + 1) * E],
                                             pg[:, :E], ACTF.Sigmoid)
                for mg in range(NB * MSUB):
                    for n in range(2):
                        pYs = py.tile([P, 512], F32, name="pys", tag="py")
                        for si in range(SICT):
                            nc.tensor.matmul(
                                pYs[:, :],
                                lhsT=As[:, si, mg * P:(mg + 1) * P],
                                rhs=swd_sb[:, si, n * 512:(n + 1) * 512],
                                start=(si == 0), stop=(si == SICT - 1))
                        nc.scalar.activation(ys[:, mg, n, :], pYs[:, :],
                                             ACTF.Copy)

            # ------------- routing -------------
            sfc = rpool.tile([P, P], F32, name="sfc")
            nc.vector.tensor_tensor(sfc[:], scores[:], bias_sb[:], op=ALU.add)
            v4 = sfc[:].rearrange("p (t g e) -> p t g e", t=8, g=4, e=4)

            def t32(nm):
                return rpool.tile([P, 32], F32, name=nm)

            def v32(t):
                return t[:].rearrange("p (t g) -> p t g", t=8)

            a_, b_, c_, d_ = (v4[:, :, :, j] for j in range(4))
            m1, n1, m2, n2 = t32("m1"), t32("n1"), t32("m2"), t32("n2")
            top1, t3, t4, sec, gs = (t32(x) for x in
                                     ("top1", "t3", "t4", "sec", "gs"))
            nc.vector.tensor_tensor(v32(m1), a_, b_, op=ALU.max)
            nc.vector.tensor_tensor(v32(n1), a_, b_, op=ALU.min)
            nc.vector.tensor_tensor(v32(m2), c_, d_, op=ALU.max)
            nc.vector.tensor_tensor(v32(n2), c_, d_, op=ALU.min)
            nc.vector.tensor_tensor(top1[:], m1[:], m2[:], op=ALU.max)
            nc.vector.tensor_tensor(t3[:], m1[:], m2[:], op=ALU.min)
            nc.vector.tensor_tensor(t4[:], n1[:], n2[:], op=ALU.max)
            nc.vector.tensor_tensor(sec[:], t3[:], t4[:], op=ALU.max)
            nc.vector.tensor_tensor(gs[:], top1[:], sec[:], op=ALU.add)

            gv = gs[:].rearrange("p (t g) -> p t g", t=8)

            def t8(nm):
                return rpool.tile([P, 8], F32, name=nm)

            u1, l1, u2, l2, q1, q2, thr = (t8(x) for x in
                                           ("u1", "l1", "u2", "l2", "q1", "q2",
                                            "thr"))
            x0, x1, x2, x3 = (gv[:, :, j] for j in range(4))
            nc.vector.tensor_tensor(u1[:], x0, x1, op=ALU.max)
            nc.vector.tensor_tensor(l1[:], x0, x1, op=ALU.min)
            nc.vector.tensor_tensor(u2[:], x2, x3, op=ALU.max)
            nc.vector.tensor_tensor(l2[:], x2, x3, op=ALU.min)
            nc.vector.tensor_tensor(q1[:], u1[:], u2[:], op=ALU.min)
            nc.vector.tensor_tensor(q2[:], l1[:], l2[:], op=ALU.max)
            nc.vector.tensor_tensor(thr[:], q1[:], q2[:], op=ALU.max)

            pen = t32("pen")
            thrb = thr[:].rearrange("p (t o) -> p t o", o=1) \
                .broadcast_to([P, 8, 4])
            nc.vector.tensor_tensor(v32(pen), gv, thrb, op=ALU.is_lt)
            nc.vector.tensor_scalar_mul(pen[:], pen[:], BIG)

            masked = rpool.tile([P, P], F32, name="masked")
            mv4 = masked[:].rearrange("p (t g e) -> p t g e", t=8, g=4, e=4)
            penb = pen[:].rearrange("p (t g o) -> p t g o", t=8, o=1) \
                .broadcast_to([P, 8, 4, 4])
            nc.vector.scalar_tensor_tensor(mv4, v4, OFF, penb,
                                           op0=ALU.add, op1=ALU.subtract)

            mv3 = masked[:].rearrange("p (t e) -> p t e", t=8)
            mx = t8("mx")
            lt = rpool.tile([P, P], F32, name="lt")
            lt3 = lt[:].rearrange("p (t e) -> p t e", t=8)
            for _ in range(6):
                nc.vector.tensor_reduce(mx[:], mv3, axis=mybir.AxisListType.X,
                                        op=ALU.max)
                mxb = mx[:].rearrange("p (t o) -> p t o", o=1) \
                    .broadcast_to([P, 8, 16])
                nc.vector.tensor_tensor(lt3, mv3, mxb, op=ALU.is_lt)
                nc.vector.tensor_tensor(masked[:], lt[:], masked[:],
                                        op=ALU.mult)

            sel = rpool.tile([P, P], F32, name="sel")
            nc.vector.tensor_scalar(sel[:], masked[:], 0.0, None,
                                    op0=ALU.is_equal)
            sw = rpool.tile([P, P], F32, name="swt")
            nc.vector.tensor_tensor(sw[:], scores[:], sel[:], op=ALU.mult)
            sums = t8("sums")
            nc.vector.tensor_reduce(sums[:],
                                    sw[:].rearrange("p (t e) -> p t e", t=8),
                                    axis=mybir.AxisListType.X, op=ALU.add)
            rec = t8("rec")
            nc.vector.reciprocal(rec[:], sums[:])
            cw = [rpool.tile([P, 8], F32, name=f"cw{e}") for e in range(2)]
            swv = sw[:].rearrange("p (t e) -> p t e", t=8)
            for e in range(2):
                for tt in range(8):
                    nc.vector.scalar_tensor_tensor(
                        cw[e][:, tt:tt + 1], swv[:, tt, e:e + 1], 2.0,
                        rec[:, tt:tt + 1], op0=ALU.mult, op1=ALU.mult)

            # ------------- DRAM partials & collectives -------------
            partial = [dram.tile([T, 512], F32, name=f"partial{n}")
                       for n in range(2)]
            rs = [dram.tile([P, 512], F32, name=f"rs{n}") for n in range(2)]

            # ------------- routed experts -------------
            for rep, b in [(rep, b) for rep in range(routed_reps)
                           for b in range(NB)]:
                last_rep = rep == routed_reps - 1
                tsl = slice(b * BLK, (b + 1) * BLK)
                A = []
                for e in range(2):
                    At = apool.tile([P, IT, BLK], BF16, name=f"A{e}",
                                    tag=f"A{e}")
                    nc.vector.memset(At[P - 64:, IT - 1, :], 0.0)
                    for i in range(IT):
                        ip = P if i < IT - 1 else I - (IT - 1) * P
                        pG = pgu.tile([P, 512], F32, name="pgu", tag="pgu")
                        pU = pgu.tile([P, 512], F32, name="pgu2", tag="pgu")
                        for k in range(KT):
                            nc.tensor.matmul(
                                pG[:ip, :],
                                lhsT=wg_sb[e][:, k, i * P:i * P + ip],
                                rhs=ht_sb[:, k, tsl],
                                start=(k == 0), stop=(k == KT - 1))
                        for k in range(KT):
                            nc.tensor.matmul(
                                pU[:ip, :],
                                lhsT=wu_sb[e][:, k, i * P:i * P + ip],
                                rhs=ht_sb[:, k, tsl],
                                start=(k == 0), stop=(k == KT - 1))
                        st = stmp.tile([P, BLK], F32, name="st", tag="st")
                        nc.scalar.activation(st[:ip, :], pG[:ip, :], ACTF.Silu)
                        nc.vector.tensor_tensor(At[:ip, i, :], st[:ip, :],
                                                pU[:ip, :], op=ALU.mult)
                    A.append(At)

                for n in range(2):
                    pt = part.tile([P, MSUB, 512], F32, name="pt", tag="pt")
                    for e in range(2):
                        for m in range(MSUB):
                            pY = py.tile([P, 512], F32, name="py", tag="py")
                            for i in range(IT):
                                nc.tensor.matmul(
                                    pY[:, :],
                                    lhsT=A[e][:, i, m * P:(m + 1) * P],
                                    rhs=wd_sb[e][:, i, n * 512:(n + 1) * 512],
                                    start=(i == 0), stop=(i == IT - 1))
                            tt = b * MSUB + m
                            if e == 0:
                                # seed with the shared-expert partial
                                nc.vector.scalar_tensor_tensor(
                                    pt[:, m, :], pY[:, :], cw[0][:, tt:tt + 1],
                                    ys[:, tt, n, :], op0=ALU.mult, op1=ALU.add)
                            else:
                                nc.vector.scalar_tensor_tensor(
                                    pt[:, m, :], pY[:, :], cw[1][:, tt:tt + 1],
                                    pt[:, m, :], op0=ALU.mult, op1=ALU.add)
                    if last_rep:
                        for m in range(MSUB):
                            r0 = b * BLK + m * P
                            nc.sync.dma_start(
                                out=partial[n][r0:r0 + P, :],
                                in_=pt[:, m, :])
                    if last_rep and b == NB - 1 and with_collective:
                        for _cr in range(coll_reps):
                            nc.gpsimd.collective_compute(
                                "ReduceScatter", ALU.add,
                                replica_groups=[list(range(NCORES))],
                                ins=[partial[n][:]], outs=[rs[n][:]])

            # ------------- epilogue -------------
            for n in range(2):
                if with_collective:
                    nc.sync.dma_start(out=out[:, n * 512:(n + 1) * 512],
                                      in_=rs[n][:])
                else:
                    nc.sync.dma_start(out=out[:, n * 512:(n + 1) * 512],
                                      in_=partial[n][0:P, :])

    _split_sync_waits(nc)
    return nc


def _perm_for_core(c):
    g_sel = c >> 1
    rot = 2 * (c & 1)
    perm = [4 * g_sel + ((rot + j) % 4) for j in range(4)]
    for g in range(4):
        if g != g_sel:
            perm.extend(range(4 * g, 4 * g + 4))
    return perm


def prepare_in_maps(h, gate_w, bias, wg, wu, wd, swg, swu, swd):
    bf = ml_dtypes.bfloat16
    h = np.asarray(h, np.float32)
    gate_w = np.asarray(gate_w, np.float32)
    bias = np.asarray(bias, np.float32)

    ht32 = np.ascontiguousarray(h.T)                      # [H, T] f32
    # blocked gate operand: [tt, p, k, t], 4KB contiguous per partition row
    ht32b = np.ascontiguousarray(
        ht32.reshape(KT, P, 8, P).transpose(2, 1, 0, 3))
    ht = ht32.astype(bf)                                  # [H, T] bf16
    gwt = np.ascontiguousarray(gate_w.T)                  # [H, E] f32

    swg32 = np.asarray(swg, np.float32)
    swu32 = np.asarray(swu, np.float32)
    swd32 = np.asarray(swd, np.float32)

    wd_pad = np.zeros((E, IPAD, H), np.float32)
    wd_pad[:, :I, :] = np.asarray(wd, np.float32)

    in_maps = []
    for c in range(NCORES):
        e0, e1 = 2 * c, 2 * c + 1
        perm = _perm_for_core(c)
        csl = slice(c * SIC, (c + 1) * SIC)
        in_maps.append({
            "ht": ht,
            "ht32": ht32b,
            "gw32": np.ascontiguousarray(gwt[:, perm]),
            "bias_rep": np.tile(bias[perm], (P, 8)).astype(np.float32),
            "wg0": np.asarray(wg[e0], np.float32).astype(bf),
            "wu0": np.asarray(wu[e0], np.float32).astype(bf),
            "wg1": np.asarray(wg[e1], np.float32).astype(bf),
            "wu1": np.asarray(wu[e1], np.float32).astype(bf),
            "wd0": wd_pad[e0].astype(bf),
            "wd1": wd_pad[e1].astype(bf),
            "swg_my": np.ascontiguousarray(swg32[:, csl]).astype(bf),
            "swu_my": np.ascontiguousarray(swu32[:, csl]).astype(bf),
            "swd_my": np.ascontiguousarray(swd32[csl, :]).astype(bf),
        })

    return in_maps


def get_nc(**kw):
    key = tuple(sorted(kw.items()))
    if key not in _BUILD_CACHE:
        _BUILD_CACHE[key] = _build(**kw)
    return _BUILD_CACHE[key]


def kernel(h, gate_w, bias, wg, wu, wd, swg, swu, swd):
    in_maps = prepare_in_maps(h, gate_w, bias, wg, wu, wd, swg, swu, swd)
    res = run_bass_kernel_spmd(get_nc(), in_maps, list(range(NCORES)))
    return np.concatenate([res.results[c]["out"] for c in range(NCORES)],
                          axis=0).astype(np.float32)



# revision 6
# speedup vs baseline: 1.1154x; 1.1154x over previous
"""DeepSeek-V2-style MoE kernel for 8 Trainium2 NeuronCores.

Strategy (expert-parallel, dense):
- 16 experts, 8 cores -> 2 experts per core. Each core computes its two
  experts' SwiGLU MLPs densely over all 1024 tokens (bf16 matmuls, fp32
  accumulate), weighted by on-device routing weights.
- The shared expert is sharded over its intermediate dim (256 of 2048 per
  core) across all tokens; its per-core partial seeds the routed combine,
  so one ReduceScatter(add) of the [T, H] partial (as two [T, 512] column
  halves) produces each core's final 128-token output shard directly.
- The gate (sigmoid + grouped top-k) runs on every core in fp32 (matmul
  included) so expert selection matches the fp32 reference exactly. The
  expert axis is permuted per core (group structure preserved) so each
  core's own experts sit at positions 0 and 1 -> identical SPMD program.
"""

import os
import sys

import numpy as np
import ml_dtypes

for _p in ("/opt/trn_rl_repo", os.path.expanduser("~/.axon_site/_ro/trn_rl_repo")):
    if os.path.isdir(_p) and _p not in sys.path:
        sys.path.append(_p)

import concourse.bass as bass
import concourse.mybir as mybir
import concourse.tile as tile
from concourse.bass_utils import run_bass_kernel_spmd

# problem sizes (fixed)
T, H, E, I, SI = 1024, 1024, 16, 704, 2048
P = 128
NCORES = 8
KT = H // P            # 8 contraction tiles over H
IT = 6                 # ceil(704/128) I tiles; last is 64 rows (wd zero-padded)
IPAD = IT * P          # 768
SIC = SI // NCORES     # 256: shared-expert intermediate slice per core
SICT = SIC // P        # 2
NB = 2                 # token blocks
BLK = T // NB          # 512
MSUB = BLK // P        # 4 token subtiles per block
BIG = 1.0e6
OFF = 10.0             # offset making all valid masked scores positive

F32 = mybir.dt.float32
BF16 = mybir.dt.bfloat16
ALU = mybir.AluOpType
ACTF = mybir.ActivationFunctionType

_BUILD_CACHE = {}


def _split_sync_waits(nc):
    """This walrus build allows one sync wait per instruction; move extra
    waits onto same-engine pure-wait carriers placed immediately before."""
    n_split = 0
    for f in nc.m.functions:
        for bb in f.blocks:
            out = []
            for ins in bb.instructions:
                si = ins.sync_info
                if si is not None and si.on_wait and len(si.on_wait) > 1:
                    waits = list(si.on_wait)
                    head, tail = waits[:-1], waits[-1:]
                    for i, w in enumerate(head):
                        carrier = mybir.InstEventSemaphore(
                            name=f"{ins.name}-ws{i}",
                            engine=ins.engine,
                            ins=[],
                            outs=[],
                            sync_info=mybir.SyncInfo(on_wait=[w], on_update=[]),
                        )
                        nc.register_instruction(carrier, overwrite=True)
                        out.append(carrier)
                    ins.sync_info = mybir.SyncInfo(on_wait=tail,
                                                   on_update=si.on_update)
                    n_split += 1
                out.append(ins)
            bb.instructions[:] = out
    return nc


def _build(with_collective=True, routed_reps=1, shared_reps=1, coll_reps=1,
           nq=4):
    nc = bass.Bass(num_devices=NCORES)

    # ---- parameters (per-core contents supplied host-side) ----
    ht = nc.declare_dram_parameter("ht", [H, T], BF16, isOutput=False)
    ht32 = nc.declare_dram_parameter("ht32", [8, P, KT, P], F32,
                                     isOutput=False)
    gw32 = nc.declare_dram_parameter("gw32", [H, E], F32, isOutput=False)
    bias_rep = nc.declare_dram_parameter("bias_rep", [P, P], F32, isOutput=False)
    wgu = [[nc.declare_dram_parameter(f"w{n}{e}", [H, I], BF16, isOutput=False)
            for n in ("g", "u")] for e in range(2)]
    wdp = [nc.declare_dram_parameter(f"wd{e}", [IPAD, H], BF16, isOutput=False)
           for e in range(2)]
    swg_my = nc.declare_dram_parameter("swg_my", [H, SIC], BF16, isOutput=False)
    swu_my = nc.declare_dram_parameter("swu_my", [H, SIC], BF16, isOutput=False)
    swd_my = nc.declare_dram_parameter("swd_my", [SIC, H], BF16, isOutput=False)
    out = nc.declare_dram_parameter("out", [P, H], F32, isOutput=True)

    with tile.TileContext(nc) as tc:
        with (
            tc.tile_pool(name="const", bufs=1) as const,
            tc.tile_pool(name="ht32s", bufs=1) as ht32s,
            tc.tile_pool(name="wpool", bufs=1) as wpool,
            tc.tile_pool(name="apool", bufs=1) as apool,
            tc.tile_pool(name="stmp", bufs=2) as stmp,
            tc.tile_pool(name="part", bufs=4) as part,
            tc.tile_pool(name="rpool", bufs=1) as rpool,
            tc.tile_pool(name="pgu", bufs=4, space="PSUM") as pgu,
            tc.tile_pool(name="py", bufs=4, space="PSUM") as py,
            tc.tile_pool(name="dram", bufs=1, space="DRAM") as dram,
        ):
            # ------------- gate operand loads (gate runs after shared G/U) --
            gw_sb = const.tile([P, KT, E], F32, name="gw_sb")
            nc.sync.dma_start(out=gw_sb[:],
                              in_=gw32.rearrange("(k p) e -> p k e", p=P))
            # ------------- constant + weight loads -------------
            ht_sb = const.tile([P, KT, T], BF16, name="ht_sb")
            for k in range(KT):
                nc.sync.dma_start(out=ht_sb[:, k, :],
                                  in_=ht[k * P:(k + 1) * P, :])
            bias_sb = const.tile([P, P], F32, name="bias_sb")
            nc.sync.dma_start(out=bias_sb[:], in_=bias_rep[:])

            swg_sb = wpool.tile([P, KT, SIC], BF16, name="swg_sb", tag="swg")
            swu_sb = wpool.tile([P, KT, SIC], BF16, name="swu_sb", tag="swu")
            nc.scalar.dma_start(out=swg_sb[:],
                                in_=swg_my.rearrange("(k p) c -> p k c", p=P))
            nc.scalar.dma_start(out=swu_sb[:],
                                in_=swu_my.rearrange("(k p) c -> p k c", p=P))
            swd_sb = wpool.tile([P, SICT, H], BF16, name="swd_sb", tag="swd")
            nc.scalar.dma_start(out=swd_sb[:],
                                in_=swd_my.rearrange("(i p) h -> p i h", p=P))

            scores = rpool.tile([P, P], F32, name="scores")
            hts_t = []
            _eng = [nc.sync, nc.scalar, nc.gpsimd]
            for tt in range(8):
                hts = ht32s.tile([P, KT, P], F32, name=f"hts{tt}",
                                 tag=f"hts{tt}")
                _eng[tt % 3].dma_start(out=hts[:], in_=ht32[tt])
                hts_t.append(hts)

            wg_sb, wu_sb, wd_sb = [], [], []
            for e in range(2):
                g_t = wpool.tile([P, KT, I], BF16, name=f"wg{e}_sb", tag=f"wg{e}")
                u_t = wpool.tile([P, KT, I], BF16, name=f"wu{e}_sb", tag=f"wu{e}")
                for k in range(KT):
                    nc.sync.dma_start(out=g_t[:, k, :],
                                      in_=wgu[e][0][k * P:(k + 1) * P, :])
                    nc.sync.dma_start(out=u_t[:, k, :],
                                      in_=wgu[e][1][k * P:(k + 1) * P, :])
                d_t = wpool.tile([P, IT, H], BF16, name=f"wd{e}_sb", tag=f"wd{e}")
                for i in range(IT):
                    nc.sync.dma_start(out=d_t[:, i, :],
                                      in_=wdp[e][i * P:(i + 1) * P, :])
                wg_sb.append(g_t)
                wu_sb.append(u_t)
                wd_sb.append(d_t)


            # ------------- shared expert (intermediate slice, all tokens) --
            As = const.tile([P, SICT, T], BF16, name="As_sh")
            ys = const.tile([P, NB * MSUB, 2, 512], BF16, name="ys")
            for rep_s in range(shared_reps):
                for si in range(SICT):
                    for b in range(NB):
                        tsl = slice(b * BLK, (b + 1) * BLK)
                        pGs = pgu.tile([P, 512], F32, name="pgs", tag="pgu")
                        pUs = pgu.tile([P, 512], F32, name="pus", tag="pgu")
                        for k in range(KT):
                            nc.tensor.matmul(
                                pGs[:, :], lhsT=swg_sb[:, k, si * P:(si + 1) * P],
                                rhs=ht_sb[:, k, tsl],
                                start=(k == 0), stop=(k == KT - 1))
                        for k in range(KT):
                            nc.tensor.matmul(
                                pUs[:, :], lhsT=swu_sb[:, k, si * P:(si + 1) * P],
                                rhs=ht_sb[:, k, tsl],
                                start=(k == 0), stop=(k == KT - 1))
                        sts = stmp.tile([P, BLK], F32, name="st", tag="st")
                        nc.scalar.activation(sts[:, :], pGs[:, :], ACTF.Silu)
                        nc.vector.tensor_tensor(As[:, si, tsl], sts[:, :],
                                                pUs[:, :], op=ALU.mult)
                if rep_s == 0:
                    for tt in range(8):
                        pg = pgu.tile([P, 512], F32, name="pgate", tag="pgu")
                        for k in range(KT):
                            nc.tensor.matmul(pg[:, :E],
                                             lhsT=hts_t[tt][:, k, :],
                                             rhs=gw_sb[:, k, :],
                                             start=(k == 0), stop=(k == KT - 1))
                        nc.scalar.activation(scores[:, tt * E:(tt + 1) * E],
                                             pg[:, :E], ACTF.Sigmoid)
                for mg in range(NB * MSUB):
                    for n in range(2):
                        pYs = py.tile([P, 512], F32, name="pys", tag="py")
                        for si in range(SICT):
                            nc.tensor.matmul(
                                pYs[:, :],
                                lhsT=As[:, si, mg * P:(mg + 1) * P],
                                rhs=swd_sb[:, si, n * 512:(n + 1) * 512],
                                start=(si == 0), stop=(si == SICT - 1))
                        nc.scalar.activation(ys[:, mg, n, :], pYs[:, :],
                                             ACTF.Copy)

            # ------------- routing -------------
            sfc = rpool.tile([P, P], F32, name="sfc")
            nc.vector.tensor_tensor(sfc[:], scores[:], bias_sb[:], op=ALU.add)
            v4 = sfc[:].rearrange("p (t g e) -> p t g e", t=8, g=4, e=4)

            def t32(nm):
                return rpool.tile([P, 32], F32, name=nm)

            def v32(t):
                return t[:].rearrange("p (t g) -> p t g", t=8)

            a_, b_, c_, d_ = (v4[:, :, :, j] for j in range(4))
            m1, n1, m2, n2 = t32("m1"), t32("n1"), t32("m2"), t32("n2")
            top1, t3, t4, sec, gs = (t32(x) for x in
                                     ("top1", "t3", "t4", "sec", "gs"))
            nc.vector.tensor_tensor(v32(m1), a_, b_, op=ALU.max)
            nc.vector.tensor_tensor(v32(n1), a_, b_, op=ALU.min)
            nc.vector.tensor_tensor(v32(m2), c_, d_, op=ALU.max)
            nc.vector.tensor_tensor(v32(n2), c_, d_, op=ALU.min)
            nc.vector.tensor_tensor(top1[:], m1[:], m2[:], op=ALU.max)
            nc.vector.tensor_tensor(t3[:], m1[:], m2[:], op=ALU.min)
            nc.vector.tensor_tensor(t4[:], n1[:], n2[:], op=ALU.max)
            nc.vector.tensor_tensor(sec[:], t3[:], t4[:], op=ALU.max)
            nc.vector.tensor_tensor(gs[:], top1[:], sec[:], op=ALU.add)

            gv = gs[:].rearrange("p (t g) -> p t g", t=8)

            def t8(nm):
                return rpool.tile([P, 8], F32, name=nm)

            u1, l1, u2, l2, q1, q2, thr = (t8(x) for x in
                                           ("u1", "l1", "u2", "l2", "q1", "q2",
                                            "thr"))
            x0, x1, x2, x3 = (gv[:, :, j] for j in range(4))
            nc.vector.tensor_tensor(u1[:], x0, x1, op=ALU.max)
            nc.vector.tensor_tensor(l1[:], x0, x1, op=ALU.min)
            nc.vector.tensor_tensor(u2[:], x2, x3, op=ALU.max)
            nc.vector.tensor_tensor(l2[:], x2, x3, op=ALU.min)
            nc.vector.tensor_tensor(q1[:], u1[:], u2[:], op=ALU.min)
            nc.vector.tensor_tensor(q2[:], l1[:], l2[:], op=ALU.max)
            nc.vector.tensor_tensor(thr[:], q1[:], q2[:], op=ALU.max)

            pen = t32("pen")
            thrb = thr[:].rearrange("p (t o) -> p t o", o=1) \
                .broadcast_to([P, 8, 4])
            nc.vector.tensor_tensor(v32(pen), gv, thrb, op=ALU.is_lt)
            nc.vector.tensor_scalar_mul(pen[:], pen[:], BIG)

            masked = rpool.tile([P, P], F32, name="masked")
            mv4 = masked[:].rearrange("p (t g e) -> p t g e", t=8, g=4, e=4)
            penb = pen[:].rearrange("p (t g o) -> p t g o", t=8, o=1) \
                .broadcast_to([P, 8, 4, 4])
            nc.vector.scalar_tensor_tensor(mv4, v4, OFF, penb,
                                           op0=ALU.add, op1=ALU.subtract)

            mv3 = masked[:].rearrange("p (t e) -> p t e", t=8)
            mx = t8("mx")
            lt = rpool.tile([P, P], F32, name="lt")
            lt3 = lt[:].rearrange("p (t e) -> p t e", t=8)
            for _ in range(6):
                nc.vector.tensor_reduce(mx[:], mv3, axis=mybir.AxisListType.X,
                                        op=ALU.max)
                mxb = mx[:].rearrange("p (t o) -> p t o", o=1) \
                    .broadcast_to([P, 8, 16])
                nc.vector.tensor_tensor(lt3, mv3, mxb, op=ALU.is_lt)
                nc.vector.tensor_tensor(masked[:], lt[:], masked[:],
                                        op=ALU.mult)

            sel = rpool.tile([P, P], F32, name="sel")
            nc.vector.tensor_scalar(sel[:], masked[:], 0.0, None,
                                    op0=ALU.is_equal)
            sw = rpool.tile([P, P], F32, name="swt")
            nc.vector.tensor_tensor(sw[:], scores[:], sel[:], op=ALU.mult)
            sums = t8("sums")
            nc.vector.tensor_reduce(sums[:],
                                    sw[:].rearrange("p (t e) -> p t e", t=8),
                                    axis=mybir.AxisListType.X, op=ALU.add)
            rec = t8("rec")
            nc.vector.reciprocal(rec[:], sums[:])
            cw = [rpool.tile([P, 8], F32, name=f"cw{e}") for e in range(2)]
            swv = sw[:].rearrange("p (t e) -> p t e", t=8)
            for e in range(2):
                for tt in range(8):
                    nc.vector.scalar_tensor_tensor(
                        cw[e][:, tt:tt + 1], swv[:, tt, e:e + 1], 2.0,
                        rec[:, tt:tt + 1], op0=ALU.mult, op1=ALU.mult)

            # ------------- DRAM partials & collectives -------------
            QW = H // nq  # column width per quarter
            partial = [dram.tile([T, QW], F32, name=f"partial{q}")
                       for q in range(nq)]
            rs = [dram.tile([P, QW], F32, name=f"rs{q}") for q in range(nq)]

            # ------------- routed experts -------------
            # Phase 1: A tiles (SwiGLU intermediates) for both blocks.
            for rep in range(routed_reps):
                A = {}
                for b in range(NB):
                    tsl = slice(b * BLK, (b + 1) * BLK)
                    for e in range(2):
                        At = apool.tile([P, IT, BLK], BF16, name=f"A{b}{e}",
                                        tag=f"A{b}{e}")
                        nc.vector.memset(At[P - 64:, IT - 1, :], 0.0)
                        for i in range(IT):
                            ip = P if i < IT - 1 else I - (IT - 1) * P
                            pG = pgu.tile([P, 512], F32, name="pgu", tag="pgu")
                            pU = pgu.tile([P, 512], F32, name="pgu2",
                                          tag="pgu")
                            for k in range(KT):
                                nc.tensor.matmul(
                                    pG[:ip, :],
                                    lhsT=wg_sb[e][:, k, i * P:i * P + ip],
                                    rhs=ht_sb[:, k, tsl],
                                    start=(k == 0), stop=(k == KT - 1))
                            for k in range(KT):
                                nc.tensor.matmul(
                                    pU[:ip, :],
                                    lhsT=wu_sb[e][:, k, i * P:i * P + ip],
                                    rhs=ht_sb[:, k, tsl],
                                    start=(k == 0), stop=(k == KT - 1))
                            st = stmp.tile([P, BLK], F32, name="st", tag="st")
                            nc.scalar.activation(st[:ip, :], pG[:ip, :],
                                                 ACTF.Silu)
                            nc.vector.tensor_tensor(At[:ip, i, :], st[:ip, :],
                                                    pU[:ip, :], op=ALU.mult)
                        A[b, e] = At

                # Phase 2: Y + combine per column quarter; launch each
                # quarter's ReduceScatter as soon as its partial is complete
                # so RS q overlaps quarter q+1 compute.
                last_rep = rep == routed_reps - 1
                for q in range(nq):
                    n, off = q // (nq // 2), (q % (nq // 2)) * QW
                    for b in range(NB):
                        for m in range(MSUB):
                            tt = b * MSUB + m
                            pt = part.tile([P, QW], F32, name="pt", tag="pt")
                            for e in range(2):
                                pY = py.tile([P, 512], F32, name="py",
                                             tag="py")
                                for i in range(IT):
                                    nc.tensor.matmul(
                                        pY[:, :QW],
                                        lhsT=A[b, e][:, i, m * P:(m + 1) * P],
                                        rhs=wd_sb[e][:, i,
                                                     n * 512 + off:
                                                     n * 512 + off + QW],
                                        start=(i == 0), stop=(i == IT - 1))
                                if e == 0:
                                    # seed with the shared-expert partial
                                    nc.vector.scalar_tensor_tensor(
                                        pt[:, :], pY[:, :QW],
                                        cw[0][:, tt:tt + 1],
                                        ys[:, tt, n, off:off + QW],
                                        op0=ALU.mult, op1=ALU.add)
                                else:
                                    nc.vector.scalar_tensor_tensor(
                                        pt[:, :], pY[:, :QW],
                                        cw[1][:, tt:tt + 1],
                                        pt[:, :], op0=ALU.mult, op1=ALU.add)
                            if last_rep:
                                r0 = b * BLK + m * P
                                nc.scalar.dma_start(
                                    out=partial[q][r0:r0 + P, :],
                                    in_=pt[:, :])
                    if last_rep and with_collective:
                        for _cr in range(coll_reps):
                            nc.gpsimd.collective_compute(
                                "ReduceScatter", ALU.add,
                                replica_groups=[list(range(NCORES))],
                                ins=[partial[q][:]], outs=[rs[q][:]])

            # ------------- epilogue -------------
            for q in range(nq):
                if with_collective:
                    nc.sync.dma_start(out=out[:, q * QW:(q + 1) * QW],
                                      in_=rs[q][:])
                else:
                    nc.sync.dma_start(out=out[:, q * QW:(q + 1) * QW],
                                      in_=partial[q][0:P, :])

    _split_sync_waits(nc)
    return nc


def _perm_for_core(c):
    g_sel = c >> 1
    rot = 2 * (c & 1)
    perm = [4 * g_sel + ((rot + j) % 4) for j in range(4)]
    for g in range(4):
        if g != g_sel:
            perm.extend(range(4 * g, 4 * g + 4))
    return perm


def prepare_in_maps(h, gate_w, bias, wg, wu, wd, swg, swu, swd):
    bf = ml_dtypes.bfloat16
    h = np.asarray(h, np.float32)
    gate_w = np.asarray(gate_w, np.float32)
    bias = np.asarray(bias, np.float32)

    ht32 = np.ascontiguousarray(h.T)                      # [H, T] f32
    # blocked gate operand: [tt, p, k, t], 4KB contiguous per partition row
    ht32b = np.ascontiguousarray(
        ht32.reshape(KT, P, 8, P).transpose(2, 1, 0, 3))
    ht = ht32.astype(bf)                                  # [H, T] bf16
    gwt = np.ascontiguousarray(gate_w.T)                  # [H, E] f32

    swg32 = np.asarray(swg, np.float32)
    swu32 = np.asarray(swu, np.float32)
    swd32 = np.asarray(swd, np.float32)

    wd_pad = np.zeros((E, IPAD, H), np.float32)
    wd_pad[:, :I, :] = np.asarray(wd, np.float32)

    in_maps = []
    for c in range(NCORES):
        e0, e1 = 2 * c, 2 * c + 1
        perm = _perm_for_core(c)
        csl = slice(c * SIC, (c + 1) * SIC)
        in_maps.append({
            "ht": ht,
            "ht32": ht32b,
            "gw32": np.ascontiguousarray(gwt[:, perm]),
            "bias_rep": np.tile(bias[perm], (P, 8)).astype(np.float32),
            "wg0": np.asarray(wg[e0], np.float32).astype(bf),
            "wu0": np.asarray(wu[e0], np.float32).astype(bf),
            "wg1": np.asarray(wg[e1], np.float32).astype(bf),
            "wu1": np.asarray(wu[e1], np.float32).astype(bf),
            "wd0": wd_pad[e0].astype(bf),
            "wd1": wd_pad[e1].astype(bf),
            "swg_my": np.ascontiguousarray(swg32[:, csl]).astype(bf),
            "swu_my": np.ascontiguousarray(swu32[:, csl]).astype(bf),
            "swd_my": np.ascontiguousarray(swd32[csl, :]).astype(bf),
        })

    return in_maps


def get_nc(**kw):
    key = tuple(sorted(kw.items()))
    if key not in _BUILD_CACHE:
        _BUILD_CACHE[key] = _build(**kw)
    return _BUILD_CACHE[key]


def kernel(h, gate_w, bias, wg, wu, wd, swg, swu, swd):
    in_maps = prepare_in_maps(h, gate_w, bias, wg, wu, wd, swg, swu, swd)
    res = run_bass_kernel_spmd(get_nc(), in_maps, list(range(NCORES)))
    return np.concatenate([res.results[c]["out"] for c in range(NCORES)],
                          axis=0).astype(np.float32)



# revision 12
# speedup vs baseline: 1.9522x; 1.7502x over previous
"""DeepSeek-V2-style MoE kernel for 8 Trainium2 NeuronCores.

Strategy (expert-parallel, dense):
- 16 experts, 8 cores -> 2 experts per core. Each core computes its two
  experts' SwiGLU MLPs densely over all 1024 tokens (bf16 matmuls, fp32
  accumulate), weighted by on-device routing weights.
- The shared expert is sharded over its intermediate dim (256 of 2048 per
  core) across all tokens; its per-core partial seeds the routed combine,
  so one ReduceScatter(add) of the [T, H] partial (as two [T, 512] column
  halves) produces each core's final 128-token output shard directly.
- The gate (sigmoid + grouped top-k) runs on every core in fp32 (matmul
  included) so expert selection matches the fp32 reference exactly. The
  expert axis is permuted per core (group structure preserved) so each
  core's own experts sit at positions 0 and 1 -> identical SPMD program.
"""

import os
import sys

import numpy as np
import ml_dtypes

for _p in ("/opt/trn_rl_repo", os.path.expanduser("~/.axon_site/_ro/trn_rl_repo")):
    if os.path.isdir(_p) and _p not in sys.path:
        sys.path.append(_p)

import concourse.bass as bass
import concourse.mybir as mybir
import concourse.tile as tile
from concourse.bass_utils import run_bass_kernel_spmd

# problem sizes (fixed)
T, H, E, I, SI = 1024, 1024, 16, 704, 2048
P = 128
NCORES = 8
KT = H // P            # 8 contraction tiles over H
IT = 6                 # ceil(704/128) I tiles per expert (Y-side view)
IT2 = 2 * I // P       # 11: both experts' intermediates packed, no padding
I2 = 2 * I             # 1408 packed intermediate rows
SIC = SI // NCORES     # 256: shared-expert intermediate slice per core
SICT = SIC // P        # 2
NB = 2                 # token blocks
BLK = T // NB          # 512
MSUB = BLK // P        # 4 token subtiles per block
BIG = 1.0e6
OFF = 10.0             # offset making all valid masked scores positive

F32 = mybir.dt.float32
BF16 = mybir.dt.bfloat16
ALU = mybir.AluOpType
ACTF = mybir.ActivationFunctionType

_BUILD_CACHE = {}


def _split_sync_waits(nc):
    """This walrus build allows one sync wait per instruction; move extra
    waits onto same-engine pure-wait carriers placed immediately before."""
    n_split = 0
    for f in nc.m.functions:
        for bb in f.blocks:
            out = []
            for ins in bb.instructions:
                si = ins.sync_info
                if si is not None and si.on_wait and len(si.on_wait) > 1:
                    waits = list(si.on_wait)
                    head, tail = waits[:-1], waits[-1:]
                    for i, w in enumerate(head):
                        carrier = mybir.InstEventSemaphore(
                            name=f"{ins.name}-ws{i}",
                            engine=ins.engine,
                            ins=[],
                            outs=[],
                            sync_info=mybir.SyncInfo(on_wait=[w], on_update=[]),
                        )
                        nc.register_instruction(carrier, overwrite=True)
                        out.append(carrier)
                    ins.sync_info = mybir.SyncInfo(on_wait=tail,
                                                   on_update=si.on_update)
                    n_split += 1
                out.append(ins)
            bb.instructions[:] = out
    return nc


def _build(with_collective=True, routed_reps=1, shared_reps=1, coll_reps=1,
           nq=4):
    nc = bass.Bass(num_devices=NCORES)

    # ---- parameters (per-core contents supplied host-side) ----
    ht = nc.declare_dram_parameter("ht", [H, T], BF16, isOutput=False)
    ht32 = nc.declare_dram_parameter("ht32", [8, P, KT, P], F32,
                                     isOutput=False)
    gw32 = nc.declare_dram_parameter("gw32", [H, E], F32, isOutput=False)
    bias_rep = nc.declare_dram_parameter("bias_rep", [P, P], F32, isOutput=False)
    wgp = nc.declare_dram_parameter("wgp", [H, I2], BF16, isOutput=False)
    wup = nc.declare_dram_parameter("wup", [H, I2], BF16, isOutput=False)
    wdk = nc.declare_dram_parameter("wdk", [I2, H], BF16, isOutput=False)
    swg_my = nc.declare_dram_parameter("swg_my", [H, SIC], BF16, isOutput=False)
    swu_my = nc.declare_dram_parameter("swu_my", [H, SIC], BF16, isOutput=False)
    swd_my = nc.declare_dram_parameter("swd_my", [SIC, H], BF16, isOutput=False)
    out = nc.declare_dram_parameter("out", [P, H], F32, isOutput=True)

    with tile.TileContext(nc) as tc:
        with (
            tc.tile_pool(name="const", bufs=1) as const,
            tc.tile_pool(name="ht32s", bufs=1) as ht32s,
            tc.tile_pool(name="wpool", bufs=1) as wpool,
            tc.tile_pool(name="apool", bufs=1) as apool,
            tc.tile_pool(name="stmp", bufs=2) as stmp,
            tc.tile_pool(name="part", bufs=4) as part,
            tc.tile_pool(name="rpool", bufs=1) as rpool,
            tc.tile_pool(name="pgu", bufs=4, space="PSUM") as pgu,
            tc.tile_pool(name="py", bufs=4, space="PSUM") as py,
            tc.tile_pool(name="dram", bufs=1, space="DRAM") as dram,
        ):
            # ------------- gate operand loads (gate runs after shared G/U) --
            gw_sb = const.tile([P, KT, E], F32, name="gw_sb")
            nc.sync.dma_start(out=gw_sb[:],
                              in_=gw32.rearrange("(k p) e -> p k e", p=P))
            # ------------- constant + weight loads -------------
            ht_sb = const.tile([P, KT, T], BF16, name="ht_sb")
            for k in range(KT):
                nc.sync.dma_start(out=ht_sb[:, k, :],
                                  in_=ht[k * P:(k + 1) * P, :])
            bias_sb = const.tile([P, P], F32, name="bias_sb")
            nc.sync.dma_start(out=bias_sb[:], in_=bias_rep[:])

            swg_sb = wpool.tile([P, KT, SIC], BF16, name="swg_sb", tag="swg")
            swu_sb = wpool.tile([P, KT, SIC], BF16, name="swu_sb", tag="swu")
            nc.scalar.dma_start(out=swg_sb[:],
                                in_=swg_my.rearrange("(k p) c -> p k c", p=P))
            nc.scalar.dma_start(out=swu_sb[:],
                                in_=swu_my.rearrange("(k p) c -> p k c", p=P))
            swd_sb = wpool.tile([P, SICT, H], BF16, name="swd_sb", tag="swd")
            nc.scalar.dma_start(out=swd_sb[:],
                                in_=swd_my.rearrange("(i p) h -> p i h", p=P))

            scores = rpool.tile([P, P], F32, name="scores")
            hts_t = []
            _eng = [nc.sync, nc.scalar, nc.gpsimd]
            for tt in range(8):
                hts = ht32s.tile([P, KT, P], F32, name=f"hts{tt}",
                                 tag=f"hts{tt}")
                _eng[tt % 3].dma_start(out=hts[:], in_=ht32[tt])
                hts_t.append(hts)

            wg_sb = wpool.tile([P, KT, I2], BF16, name="wg_sb", tag="wg")
            wu_sb = wpool.tile([P, KT, I2], BF16, name="wu_sb", tag="wu")
            for k in range(KT):
                nc.sync.dma_start(out=wg_sb[:, k, :],
                                  in_=wgp[k * P:(k + 1) * P, :])
                nc.sync.dma_start(out=wu_sb[:, k, :],
                                  in_=wup[k * P:(k + 1) * P, :])
            wd_sb = wpool.tile([P, IT2, H], BF16, name="wd_sb", tag="wd")
            for j in range(IT2):
                nc.sync.dma_start(out=wd_sb[:, j, :],
                                  in_=wdk[j * P:(j + 1) * P, :])
            # Y-side contraction segments per expert over the packed layout:
            # rows [e0 0:640 | e0 640:704 | e1 640:704 | e1 0:640]
            ysegs = [[(j, 0, P) for j in range(5)] + [(5, 0, 64)],
                     [(5, 64, P)] + [(j, 0, P) for j in range(6, IT2)]]


            # ------------- shared expert (intermediate slice, all tokens) --
            As = const.tile([P, SICT, T], BF16, name="As_sh")
            ys = const.tile([P, NB * MSUB, 2, 512], BF16, name="ys")
            for rep_s in range(shared_reps):
                for si in range(SICT):
                    for b in range(NB):
                        tsl = slice(b * BLK, (b + 1) * BLK)
                        pGs = pgu.tile([P, 512], F32, name="pgs", tag="pgu")
                        pUs = pgu.tile([P, 512], F32, name="pus", tag="pgu")
                        for k in range(KT):
                            nc.tensor.matmul(
                                pGs[:, :], lhsT=swg_sb[:, k, si * P:(si + 1) * P],
                                rhs=ht_sb[:, k, tsl],
                                start=(k == 0), stop=(k == KT - 1))
                        for k in range(KT):
                            nc.tensor.matmul(
                                pUs[:, :], lhsT=swu_sb[:, k, si * P:(si + 1) * P],
                                rhs=ht_sb[:, k, tsl],
                                start=(k == 0), stop=(k == KT - 1))
                        sts = stmp.tile([P, BLK], F32, name="st", tag="st")
                        nc.scalar.activation(sts[:, :], pGs[:, :], ACTF.Silu)
                        nc.vector.tensor_tensor(As[:, si, tsl], sts[:, :],
                                                pUs[:, :], op=ALU.mult)
                if rep_s == 0:
                    for tt in range(8):
                        pg = pgu.tile([P, 512], F32, name="pgate", tag="pgu")
                        for k in range(KT):
                            nc.tensor.matmul(pg[:, :E],
                                             lhsT=hts_t[tt][:, k, :],
                                             rhs=gw_sb[:, k, :],
                                             start=(k == 0), stop=(k == KT - 1))
                        nc.scalar.activation(scores[:, tt * E:(tt + 1) * E],
                                             pg[:, :E], ACTF.Sigmoid)
                for mg in range(NB * MSUB):
                    for n in range(2):
                        pYs = py.tile([P, 512], F32, name="pys", tag="py")
                        for si in range(SICT):
                            nc.tensor.matmul(
                                pYs[:, :],
                                lhsT=As[:, si, mg * P:(mg + 1) * P],
                                rhs=swd_sb[:, si, n * 512:(n + 1) * 512],
                                start=(si == 0), stop=(si == SICT - 1))
                        nc.scalar.activation(ys[:, mg, n, :], pYs[:, :],
                                             ACTF.Copy)

            # ------------- routing -------------
            sfc = rpool.tile([P, P], F32, name="sfc")
            nc.vector.tensor_tensor(sfc[:], scores[:], bias_sb[:], op=ALU.add)
            v4 = sfc[:].rearrange("p (t g e) -> p t g e", t=8, g=4, e=4)

            def t32(nm):
                return rpool.tile([P, 32], F32, name=nm)

            def v32(t):
                return t[:].rearrange("p (t g) -> p t g", t=8)

            a_, b_, c_, d_ = (v4[:, :, :, j] for j in range(4))
            m1, n1, m2, n2 = t32("m1"), t32("n1"), t32("m2"), t32("n2")
            top1, t3, t4, sec, gs = (t32(x) for x in
                                     ("top1", "t3", "t4", "sec", "gs"))
            nc.vector.tensor_tensor(v32(m1), a_, b_, op=ALU.max)
            nc.vector.tensor_tensor(v32(n1), a_, b_, op=ALU.min)
            nc.vector.tensor_tensor(v32(m2), c_, d_, op=ALU.max)
            nc.vector.tensor_tensor(v32(n2), c_, d_, op=ALU.min)
            nc.vector.tensor_tensor(top1[:], m1[:], m2[:], op=ALU.max)
            nc.vector.tensor_tensor(t3[:], m1[:], m2[:], op=ALU.min)
            nc.vector.tensor_tensor(t4[:], n1[:], n2[:], op=ALU.max)
            nc.vector.tensor_tensor(sec[:], t3[:], t4[:], op=ALU.max)
            nc.vector.tensor_tensor(gs[:], top1[:], sec[:], op=ALU.add)

            gv = gs[:].rearrange("p (t g) -> p t g", t=8)

            def t8(nm):
                return rpool.tile([P, 8], F32, name=nm)

            u1, l1, u2, l2, q1, q2, thr = (t8(x) for x in
                                           ("u1", "l1", "u2", "l2", "q1", "q2",
                                            "thr"))
            x0, x1, x2, x3 = (gv[:, :, j] for j in range(4))
            nc.vector.tensor_tensor(u1[:], x0, x1, op=ALU.max)
            nc.vector.tensor_tensor(l1[:], x0, x1, op=ALU.min)
            nc.vector.tensor_tensor(u2[:], x2, x3, op=ALU.max)
            nc.vector.tensor_tensor(l2[:], x2, x3, op=ALU.min)
            nc.vector.tensor_tensor(q1[:], u1[:], u2[:], op=ALU.min)
            nc.vector.tensor_tensor(q2[:], l1[:], l2[:], op=ALU.max)
            nc.vector.tensor_tensor(thr[:], q1[:], q2[:], op=ALU.max)

            pen = t32("pen")
            thrb = thr[:].rearrange("p (t o) -> p t o", o=1) \
                .broadcast_to([P, 8, 4])
            nc.vector.tensor_tensor(v32(pen), gv, thrb, op=ALU.is_lt)
            nc.vector.tensor_scalar_mul(pen[:], pen[:], BIG)

            masked = rpool.tile([P, P], F32, name="masked")
            mv4 = masked[:].rearrange("p (t g e) -> p t g e", t=8, g=4, e=4)
            penb = pen[:].rearrange("p (t g o) -> p t g o", t=8, o=1) \
                .broadcast_to([P, 8, 4, 4])
            nc.vector.scalar_tensor_tensor(mv4, v4, OFF, penb,
                                           op0=ALU.add, op1=ALU.subtract)

            mv3 = masked[:].rearrange("p (t e) -> p t e", t=8)
            mx = t8("mx")
            lt = rpool.tile([P, P], F32, name="lt")
            lt3 = lt[:].rearrange("p (t e) -> p t e", t=8)
            for _ in range(6):
                nc.vector.tensor_reduce(mx[:], mv3, axis=mybir.AxisListType.X,
                                        op=ALU.max)
                mxb = mx[:].rearrange("p (t o) -> p t o", o=1) \
                    .broadcast_to([P, 8, 16])
                nc.vector.tensor_tensor(lt3, mv3, mxb, op=ALU.is_lt)
                nc.vector.tensor_tensor(masked[:], lt[:], masked[:],
                                        op=ALU.mult)

            sel = rpool.tile([P, P], F32, name="sel")
            nc.vector.tensor_scalar(sel[:], masked[:], 0.0, None,
                                    op0=ALU.is_equal)
            sw = rpool.tile([P, P], F32, name="swt")
            nc.vector.tensor_tensor(sw[:], scores[:], sel[:], op=ALU.mult)
            sums = t8("sums")
            nc.vector.tensor_reduce(sums[:],
                                    sw[:].rearrange("p (t e) -> p t e", t=8),
                                    axis=mybir.AxisListType.X, op=ALU.add)
            rec = t8("rec")
            nc.vector.reciprocal(rec[:], sums[:])
            cw = [rpool.tile([P, 8], F32, name=f"cw{e}") for e in range(2)]
            swv = sw[:].rearrange("p (t e) -> p t e", t=8)
            for e in range(2):
                for tt in range(8):
                    nc.vector.scalar_tensor_tensor(
                        cw[e][:, tt:tt + 1], swv[:, tt, e:e + 1], 2.0,
                        rec[:, tt:tt + 1], op0=ALU.mult, op1=ALU.mult)

            # ------------- DRAM partials & collectives -------------
            QW = H // nq  # column width per quarter
            partial = [dram.tile([T, QW], F32, name=f"partial{q}")
                       for q in range(nq)]
            rs = [dram.tile([P, QW], F32, name=f"rs{q}") for q in range(nq)]

            # ------------- routed experts -------------
            # Phase 1: A tiles (SwiGLU intermediates) for both blocks.
            for rep in range(routed_reps):
                A = {}
                for b in range(NB):
                    tsl = slice(b * BLK, (b + 1) * BLK)
                    At = apool.tile([P, IT2, BLK], BF16, name=f"A{b}",
                                    tag=f"A{b}")
                    for j in range(IT2):
                        pG = pgu.tile([P, 512], F32, name="pgu", tag="pgu")
                        pU = pgu.tile([P, 512], F32, name="pgu2", tag="pgu")
                        for k in range(KT):
                            nc.tensor.matmul(
                                pG[:, :],
                                lhsT=wg_sb[:, k, j * P:(j + 1) * P],
                                rhs=ht_sb[:, k, tsl],
                                start=(k == 0), stop=(k == KT - 1))
                        for k in range(KT):
                            nc.tensor.matmul(
                                pU[:, :],
                                lhsT=wu_sb[:, k, j * P:(j + 1) * P],
                                rhs=ht_sb[:, k, tsl],
                                start=(k == 0), stop=(k == KT - 1))
                        st = stmp.tile([P, BLK], F32, name="st", tag="st")
                        nc.scalar.activation(st[:, :], pG[:, :], ACTF.Silu)
                        nc.vector.tensor_tensor(At[:, j, :], st[:, :],
                                                pU[:, :], op=ALU.mult)
                    A[b] = At

                # Phase 2: Y + combine per column quarter; launch each
                # quarter's ReduceScatter as soon as its partial is complete
                # so RS q overlaps quarter q+1 compute.
                last_rep = rep == routed_reps - 1
                for q in range(nq):
                    n, off = q // (nq // 2), (q % (nq // 2)) * QW
                    for b in range(NB):
                        for m in range(MSUB):
                            tt = b * MSUB + m
                            pt = part.tile([P, QW], F32, name="pt", tag="pt")
                            for e in range(2):
                                pY = py.tile([P, 512], F32, name="py",
                                             tag="py")
                                segs = ysegs[e]
                                for si, (j, p0, p1) in enumerate(segs):
                                    nc.tensor.matmul(
                                        pY[:, :QW],
                                        lhsT=A[b][p0:p1, j,
                                                  m * P:(m + 1) * P],
                                        rhs=wd_sb[p0:p1, j,
                                                  n * 512 + off:
                                                  n * 512 + off + QW],
                                        start=(si == 0),
                                        stop=(si == len(segs) - 1))
                                if e == 0:
                                    # seed with the shared-expert partial
                                    nc.vector.scalar_tensor_tensor(
                                        pt[:, :], pY[:, :QW],
                                        cw[0][:, tt:tt + 1],
                                        ys[:, tt, n, off:off + QW],
                                        op0=ALU.mult, op1=ALU.add)
                                else:
                                    nc.vector.scalar_tensor_tensor(
                                        pt[:, :], pY[:, :QW],
                                        cw[1][:, tt:tt + 1],
                                        pt[:, :], op0=ALU.mult, op1=ALU.add)
                            if last_rep:
                                r0 = b * BLK + m * P
                                nc.scalar.dma_start(
                                    out=partial[q][r0:r0 + P, :],
                                    in_=pt[:, :])
                    if last_rep and with_collective:
                        for _cr in range(coll_reps):
                            nc.gpsimd.collective_compute(
                                "ReduceScatter", ALU.add,
                                replica_groups=[list(range(NCORES))],
                                ins=[partial[q][:]], outs=[rs[q][:]])

            # ------------- epilogue -------------
            for q in range(nq):
                if with_collective:
                    nc.sync.dma_start(out=out[:, q * QW:(q + 1) * QW],
                                      in_=rs[q][:])
                else:
                    nc.sync.dma_start(out=out[:, q * QW:(q + 1) * QW],
                                      in_=partial[q][0:P, :])

    _split_sync_waits(nc)
    return nc


def _perm_for_core(c):
    g_sel = c >> 1
    rot = 2 * (c & 1)
    perm = [4 * g_sel + ((rot + j) % 4) for j in range(4)]
    for g in range(4):
        if g != g_sel:
            perm.extend(range(4 * g, 4 * g + 4))
    return perm


def prepare_in_maps(h, gate_w, bias, wg, wu, wd, swg, swu, swd):
    bf = ml_dtypes.bfloat16
    h = np.asarray(h, np.float32)
    gate_w = np.asarray(gate_w, np.float32)
    bias = np.asarray(bias, np.float32)

    ht32 = np.ascontiguousarray(h.T)                      # [H, T] f32
    # blocked gate operand: [tt, p, k, t], 4KB contiguous per partition row
    ht32b = np.ascontiguousarray(
        ht32.reshape(KT, P, 8, P).transpose(2, 1, 0, 3))
    ht = ht32.astype(bf)                                  # [H, T] bf16
    gwt = np.ascontiguousarray(gate_w.T)                  # [H, E] f32

    swg32 = np.asarray(swg, np.float32)
    swu32 = np.asarray(swu, np.float32)
    swd32 = np.asarray(swd, np.float32)
    wg32 = np.asarray(wg, np.float32)
    wu32 = np.asarray(wu, np.float32)
    wd32 = np.asarray(wd, np.float32)

    def pack_cols(w, e0, e1):
        # columns: [e0 0:640 | e0 640:704 | e1 640:704 | e1 0:640]
        return np.concatenate([w[e0][:, :640], w[e0][:, 640:],
                               w[e1][:, 640:], w[e1][:, :640]], axis=1)

    in_maps = []
    for c in range(NCORES):
        e0, e1 = 2 * c, 2 * c + 1
        perm = _perm_for_core(c)
        csl = slice(c * SIC, (c + 1) * SIC)
        wdk = np.concatenate([wd32[e0][:640], wd32[e0][640:],
                              wd32[e1][640:], wd32[e1][:640]], axis=0)
        in_maps.append({
            "ht": ht,
            "ht32": ht32b,
            "gw32": np.ascontiguousarray(gwt[:, perm]),
            "bias_rep": np.tile(bias[perm], (P, 8)).astype(np.float32),
            "wgp": pack_cols(wg32, e0, e1).astype(bf),
            "wup": pack_cols(wu32, e0, e1).astype(bf),
            "wdk": np.ascontiguousarray(wdk).astype(bf),
            "swg_my": np.ascontiguousarray(swg32[:, csl]).astype(bf),
            "swu_my": np.ascontiguousarray(swu32[:, csl]).astype(bf),
            "swd_my": np.ascontiguousarray(swd32[csl, :]).astype(bf),
        })

    return in_maps


def get_nc(**kw):
    key = tuple(sorted(kw.items()))
    if key not in _BUILD_CACHE:
        _BUILD_CACHE[key] = _build(**kw)
    return _BUILD_CACHE[key]


def kernel(h, gate_w, bias, wg, wu, wd, swg, swu, swd):
    in_maps = prepare_in_maps(h, gate_w, bias, wg, wu, wd, swg, swu, swd)
    res = run_bass_kernel_spmd(get_nc(), in_maps, list(range(NCORES)))
    return np.concatenate([res.results[c]["out"] for c in range(NCORES)],
                          axis=0).astype(np.float32)



# revision 15
# speedup vs baseline: 2.0518x; 1.0510x over previous
"""DeepSeek-V2-style MoE kernel for 8 Trainium2 NeuronCores.

Strategy (expert-parallel, dense):
- 16 experts, 8 cores -> 2 experts per core. Each core computes its two
  experts' SwiGLU MLPs densely over all 1024 tokens (bf16 matmuls, fp32
  accumulate), weighted by on-device routing weights.
- The shared expert is sharded over its intermediate dim (256 of 2048 per
  core) across all tokens; its per-core partial seeds the routed combine,
  so one ReduceScatter(add) of the [T, H] partial (as two [T, 512] column
  halves) produces each core's final 128-token output shard directly.
- The gate (sigmoid + grouped top-k) runs on every core in fp32 (matmul
  included) so expert selection matches the fp32 reference exactly. The
  expert axis is permuted per core (group structure preserved) so each
  core's own experts sit at positions 0 and 1 -> identical SPMD program.
"""

import os
import sys

import numpy as np
import ml_dtypes

for _p in ("/opt/trn_rl_repo", os.path.expanduser("~/.axon_site/_ro/trn_rl_repo")):
    if os.path.isdir(_p) and _p not in sys.path:
        sys.path.append(_p)

import concourse.bass as bass
import concourse.mybir as mybir
import concourse.tile as tile
from concourse.bass_utils import run_bass_kernel_spmd

# problem sizes (fixed)
T, H, E, I, SI = 1024, 1024, 16, 704, 2048
P = 128
NCORES = 8
KT = H // P            # 8 contraction tiles over H
IT = 6                 # ceil(704/128) I tiles per expert (Y-side view)
IT2 = 2 * I // P       # 11: both experts' intermediates packed, no padding
I2 = 2 * I             # 1408 packed intermediate rows
SIC = SI // NCORES     # 256: shared-expert intermediate slice per core
SICT = SIC // P        # 2
NB = 2                 # token blocks
BLK = T // NB          # 512
MSUB = BLK // P        # 4 token subtiles per block
BIG = 1.0e6
OFF = 10.0             # offset making all valid masked scores positive

F32 = mybir.dt.float32
BF16 = mybir.dt.bfloat16
ALU = mybir.AluOpType
ACTF = mybir.ActivationFunctionType

_BUILD_CACHE = {}


def _split_sync_waits(nc):
    """This walrus build allows one sync wait per instruction; move extra
    waits onto same-engine pure-wait carriers placed immediately before."""
    n_split = 0
    for f in nc.m.functions:
        for bb in f.blocks:
            out = []
            for ins in bb.instructions:
                si = ins.sync_info
                if si is not None and si.on_wait and len(si.on_wait) > 1:
                    waits = list(si.on_wait)
                    head, tail = waits[:-1], waits[-1:]
                    for i, w in enumerate(head):
                        carrier = mybir.InstEventSemaphore(
                            name=f"{ins.name}-ws{i}",
                            engine=ins.engine,
                            ins=[],
                            outs=[],
                            sync_info=mybir.SyncInfo(on_wait=[w], on_update=[]),
                        )
                        nc.register_instruction(carrier, overwrite=True)
                        out.append(carrier)
                    ins.sync_info = mybir.SyncInfo(on_wait=tail,
                                                   on_update=si.on_update)
                    n_split += 1
                out.append(ins)
            bb.instructions[:] = out
    return nc


def _build(with_collective=True, routed_reps=1, shared_reps=1, coll_reps=1,
           nq=4):
    nc = bass.Bass(num_devices=NCORES)

    # ---- parameters (per-core contents supplied host-side) ----
    ht = nc.declare_dram_parameter("ht", [H, T], BF16, isOutput=False)
    ht32 = nc.declare_dram_parameter("ht32", [8, P, KT, P], F32,
                                     isOutput=False)
    gw32 = nc.declare_dram_parameter("gw32", [H, E], F32, isOutput=False)
    bias_rep = nc.declare_dram_parameter("bias_rep", [P, P], F32, isOutput=False)
    wgp = nc.declare_dram_parameter("wgp", [H, I2], BF16, isOutput=False)
    wup = nc.declare_dram_parameter("wup", [H, I2], BF16, isOutput=False)
    wdk = nc.declare_dram_parameter("wdk", [I2, H], BF16, isOutput=False)
    swg_my = nc.declare_dram_parameter("swg_my", [H, SIC], BF16, isOutput=False)
    swu_my = nc.declare_dram_parameter("swu_my", [H, SIC], BF16, isOutput=False)
    swd_my = nc.declare_dram_parameter("swd_my", [SIC, H], BF16, isOutput=False)
    out = nc.declare_dram_parameter("out", [P, H], BF16, isOutput=True)

    with tile.TileContext(nc) as tc:
        with (
            tc.tile_pool(name="const", bufs=1) as const,
            tc.tile_pool(name="ht32s", bufs=1) as ht32s,
            tc.tile_pool(name="wpool", bufs=1) as wpool,
            tc.tile_pool(name="apool", bufs=1) as apool,
            tc.tile_pool(name="stmp", bufs=2) as stmp,
            tc.tile_pool(name="part", bufs=4) as part,
            tc.tile_pool(name="rpool", bufs=1) as rpool,
            tc.tile_pool(name="pgu", bufs=4, space="PSUM") as pgu,
            tc.tile_pool(name="py", bufs=4, space="PSUM") as py,
            tc.tile_pool(name="dram", bufs=1, space="DRAM") as dram,
        ):
            # ------------- gate operand loads (gate runs after shared G/U) --
            gw_sb = const.tile([P, KT, E], F32, name="gw_sb")
            nc.sync.dma_start(out=gw_sb[:],
                              in_=gw32.rearrange("(k p) e -> p k e", p=P))
            # ------------- constant + weight loads -------------
            ht_sb = const.tile([P, KT, T], BF16, name="ht_sb")
            for k in range(KT):
                nc.sync.dma_start(out=ht_sb[:, k, :],
                                  in_=ht[k * P:(k + 1) * P, :])
            bias_sb = const.tile([P, P], F32, name="bias_sb")
            nc.sync.dma_start(out=bias_sb[:], in_=bias_rep[:])

            swg_sb = wpool.tile([P, KT, SIC], BF16, name="swg_sb", tag="swg")
            swu_sb = wpool.tile([P, KT, SIC], BF16, name="swu_sb", tag="swu")
            nc.scalar.dma_start(out=swg_sb[:],
                                in_=swg_my.rearrange("(k p) c -> p k c", p=P))
            nc.scalar.dma_start(out=swu_sb[:],
                                in_=swu_my.rearrange("(k p) c -> p k c", p=P))
            swd_sb = wpool.tile([P, SICT, H], BF16, name="swd_sb", tag="swd")
            nc.scalar.dma_start(out=swd_sb[:],
                                in_=swd_my.rearrange("(i p) h -> p i h", p=P))

            scores = rpool.tile([P, P], F32, name="scores")
            hts_t = []
            _eng = [nc.sync, nc.scalar, nc.gpsimd]
            for tt in range(8):
                hts = ht32s.tile([P, KT, P], F32, name=f"hts{tt}",
                                 tag=f"hts{tt}")
                _eng[tt % 3].dma_start(out=hts[:], in_=ht32[tt])
                hts_t.append(hts)

            wg_sb = wpool.tile([P, KT, I2], BF16, name="wg_sb", tag="wg")
            wu_sb = wpool.tile([P, KT, I2], BF16, name="wu_sb", tag="wu")
            for k in range(KT):
                nc.sync.dma_start(out=wg_sb[:, k, :],
                                  in_=wgp[k * P:(k + 1) * P, :])
                nc.sync.dma_start(out=wu_sb[:, k, :],
                                  in_=wup[k * P:(k + 1) * P, :])
            wd_sb = wpool.tile([P, IT2, H], BF16, name="wd_sb", tag="wd")
            for j in range(IT2):
                nc.sync.dma_start(out=wd_sb[:, j, :],
                                  in_=wdk[j * P:(j + 1) * P, :])
            # Y-side contraction segments per expert over the packed layout:
            # rows [e0 0:640 | e0 640:704 | e1 640:704 | e1 0:640]
            ysegs = [[(j, 0, P) for j in range(5)] + [(5, 0, 64)],
                     [(5, 64, P)] + [(j, 0, P) for j in range(6, IT2)]]


            # ------------- shared expert (intermediate slice, all tokens) --
            As = const.tile([P, SICT, T], BF16, name="As_sh")
            ys = const.tile([P, NB * MSUB, 2, 512], BF16, name="ys")
            for rep_s in range(shared_reps):
                for si in range(SICT):
                    for b in range(NB):
                        tsl = slice(b * BLK, (b + 1) * BLK)
                        pGs = pgu.tile([P, 512], F32, name="pgs", tag="pgu")
                        pUs = pgu.tile([P, 512], F32, name="pus", tag="pgu")
                        for k in range(KT):
                            nc.tensor.matmul(
                                pGs[:, :], lhsT=swg_sb[:, k, si * P:(si + 1) * P],
                                rhs=ht_sb[:, k, tsl],
                                start=(k == 0), stop=(k == KT - 1))
                        for k in range(KT):
                            nc.tensor.matmul(
                                pUs[:, :], lhsT=swu_sb[:, k, si * P:(si + 1) * P],
                                rhs=ht_sb[:, k, tsl],
                                start=(k == 0), stop=(k == KT - 1))
                        sts = stmp.tile([P, BLK], F32, name="st", tag="st")
                        nc.scalar.activation(sts[:, :], pGs[:, :], ACTF.Silu)
                        nc.vector.tensor_tensor(As[:, si, tsl], sts[:, :],
                                                pUs[:, :], op=ALU.mult)
                if rep_s == 0:
                    for tt in range(8):
                        pg = pgu.tile([P, 512], F32, name="pgate", tag="pgu")
                        for k in range(KT):
                            nc.tensor.matmul(pg[:, :E],
                                             lhsT=hts_t[tt][:, k, :],
                                             rhs=gw_sb[:, k, :],
                                             start=(k == 0), stop=(k == KT - 1))
                        nc.scalar.activation(scores[:, tt * E:(tt + 1) * E],
                                             pg[:, :E], ACTF.Sigmoid)
                for mg in range(NB * MSUB):
                    for n in range(2):
                        pYs = py.tile([P, 512], F32, name="pys", tag="py")
                        for si in range(SICT):
                            nc.tensor.matmul(
                                pYs[:, :],
                                lhsT=As[:, si, mg * P:(mg + 1) * P],
                                rhs=swd_sb[:, si, n * 512:(n + 1) * 512],
                                start=(si == 0), stop=(si == SICT - 1))
                        nc.scalar.activation(ys[:, mg, n, :], pYs[:, :],
                                             ACTF.Copy)

            # ------------- routing -------------
            sfc = rpool.tile([P, P], F32, name="sfc")
            nc.vector.tensor_tensor(sfc[:], scores[:], bias_sb[:], op=ALU.add)
            v4 = sfc[:].rearrange("p (t g e) -> p t g e", t=8, g=4, e=4)

            def t32(nm):
                return rpool.tile([P, 32], F32, name=nm)

            def v32(t):
                return t[:].rearrange("p (t g) -> p t g", t=8)

            a_, b_, c_, d_ = (v4[:, :, :, j] for j in range(4))
            m1, n1, m2, n2 = t32("m1"), t32("n1"), t32("m2"), t32("n2")
            top1, t3, t4, sec, gs = (t32(x) for x in
                                     ("top1", "t3", "t4", "sec", "gs"))
            nc.vector.tensor_tensor(v32(m1), a_, b_, op=ALU.max)
            nc.vector.tensor_tensor(v32(n1), a_, b_, op=ALU.min)
            nc.vector.tensor_tensor(v32(m2), c_, d_, op=ALU.max)
            nc.vector.tensor_tensor(v32(n2), c_, d_, op=ALU.min)
            nc.vector.tensor_tensor(top1[:], m1[:], m2[:], op=ALU.max)
            nc.vector.tensor_tensor(t3[:], m1[:], m2[:], op=ALU.min)
            nc.vector.tensor_tensor(t4[:], n1[:], n2[:], op=ALU.max)
            nc.vector.tensor_tensor(sec[:], t3[:], t4[:], op=ALU.max)
            nc.vector.tensor_tensor(gs[:], top1[:], sec[:], op=ALU.add)

            gv = gs[:].rearrange("p (t g) -> p t g", t=8)

            def t8(nm):
                return rpool.tile([P, 8], F32, name=nm)

            u1, l1, u2, l2, q1, q2, thr = (t8(x) for x in
                                           ("u1", "l1", "u2", "l2", "q1", "q2",
                                            "thr"))
            x0, x1, x2, x3 = (gv[:, :, j] for j in range(4))
            nc.vector.tensor_tensor(u1[:], x0, x1, op=ALU.max)
            nc.vector.tensor_tensor(l1[:], x0, x1, op=ALU.min)
            nc.vector.tensor_tensor(u2[:], x2, x3, op=ALU.max)
            nc.vector.tensor_tensor(l2[:], x2, x3, op=ALU.min)
            nc.vector.tensor_tensor(q1[:], u1[:], u2[:], op=ALU.min)
            nc.vector.tensor_tensor(q2[:], l1[:], l2[:], op=ALU.max)
            nc.vector.tensor_tensor(thr[:], q1[:], q2[:], op=ALU.max)

            pen = t32("pen")
            thrb = thr[:].rearrange("p (t o) -> p t o", o=1) \
                .broadcast_to([P, 8, 4])
            nc.vector.tensor_tensor(v32(pen), gv, thrb, op=ALU.is_lt)
            nc.vector.tensor_scalar_mul(pen[:], pen[:], BIG)

            masked = rpool.tile([P, P], F32, name="masked")
            mv4 = masked[:].rearrange("p (t g e) -> p t g e", t=8, g=4, e=4)
            penb = pen[:].rearrange("p (t g o) -> p t g o", t=8, o=1) \
                .broadcast_to([P, 8, 4, 4])
            nc.vector.scalar_tensor_tensor(mv4, v4, OFF, penb,
                                           op0=ALU.add, op1=ALU.subtract)

            mv3 = masked[:].rearrange("p (t e) -> p t e", t=8)
            mx = t8("mx")
            lt = rpool.tile([P, P], F32, name="lt")
            lt3 = lt[:].rearrange("p (t e) -> p t e", t=8)
            for _ in range(6):
                nc.vector.tensor_reduce(mx[:], mv3, axis=mybir.AxisListType.X,
                                        op=ALU.max)
                mxb = mx[:].rearrange("p (t o) -> p t o", o=1) \
                    .broadcast_to([P, 8, 16])
                nc.vector.tensor_tensor(lt3, mv3, mxb, op=ALU.is_lt)
                nc.vector.tensor_tensor(masked[:], lt[:], masked[:],
                                        op=ALU.mult)

            sel = rpool.tile([P, P], F32, name="sel")
            nc.vector.tensor_scalar(sel[:], masked[:], 0.0, None,
                                    op0=ALU.is_equal)
            sw = rpool.tile([P, P], F32, name="swt")
            nc.vector.tensor_tensor(sw[:], scores[:], sel[:], op=ALU.mult)
            sums = t8("sums")
            nc.vector.tensor_reduce(sums[:],
                                    sw[:].rearrange("p (t e) -> p t e", t=8),
                                    axis=mybir.AxisListType.X, op=ALU.add)
            rec = t8("rec")
            nc.vector.reciprocal(rec[:], sums[:])
            cw = [rpool.tile([P, 8], F32, name=f"cw{e}") for e in range(2)]
            swv = sw[:].rearrange("p (t e) -> p t e", t=8)
            for e in range(2):
                for tt in range(8):
                    nc.vector.scalar_tensor_tensor(
                        cw[e][:, tt:tt + 1], swv[:, tt, e:e + 1], 2.0,
                        rec[:, tt:tt + 1], op0=ALU.mult, op1=ALU.mult)

            # ------------- DRAM partials & collectives -------------
            QW = H // nq  # column width per quarter
            # bf16 partials/collective: halves RS wire bytes + partial DMAs
            partial = [dram.tile([T, QW], BF16, name=f"partial{q}")
                       for q in range(nq)]
            rs = [dram.tile([P, QW], BF16, name=f"rs{q}") for q in range(nq)]

            # ------------- routed experts -------------
            # Phase 1: A tiles (SwiGLU intermediates) for both blocks.
            for rep in range(routed_reps):
                A = {}
                for b in range(NB):
                    tsl = slice(b * BLK, (b + 1) * BLK)
                    At = apool.tile([P, IT2, BLK], BF16, name=f"A{b}",
                                    tag=f"A{b}")
                    for j in range(IT2):
                        pG = pgu.tile([P, 512], F32, name="pgu", tag="pgu")
                        pU = pgu.tile([P, 512], F32, name="pgu2", tag="pgu")
                        for k in range(KT):
                            nc.tensor.matmul(
                                pG[:, :],
                                lhsT=wg_sb[:, k, j * P:(j + 1) * P],
                                rhs=ht_sb[:, k, tsl],
                                start=(k == 0), stop=(k == KT - 1))
                        for k in range(KT):
                            nc.tensor.matmul(
                                pU[:, :],
                                lhsT=wu_sb[:, k, j * P:(j + 1) * P],
                                rhs=ht_sb[:, k, tsl],
                                start=(k == 0), stop=(k == KT - 1))
                        st = stmp.tile([P, BLK], F32, name="st", tag="st")
                        nc.scalar.activation(st[:, :], pG[:, :], ACTF.Silu)
                        nc.vector.tensor_tensor(At[:, j, :], st[:, :],
                                                pU[:, :], op=ALU.mult)
                    A[b] = At

                # Phase 2: Y + combine per column quarter; launch each
                # quarter's ReduceScatter as soon as its partial is complete
                # so RS q overlaps quarter q+1 compute.
                last_rep = rep == routed_reps - 1
                for q in range(nq):
                    n, off = q // (nq // 2), (q % (nq // 2)) * QW
                    for b in range(NB):
                        for m in range(MSUB):
                            tt = b * MSUB + m
                            pt = part.tile([P, QW], BF16, name="pt", tag="pt")
                            for e in range(2):
                                pY = py.tile([P, 512], F32, name="py",
                                             tag="py")
                                segs = ysegs[e]
                                for si, (j, p0, p1) in enumerate(segs):
                                    nc.tensor.matmul(
                                        pY[:, :QW],
                                        lhsT=A[b][p0:p1, j,
                                                  m * P:(m + 1) * P],
                                        rhs=wd_sb[p0:p1, j,
                                                  n * 512 + off:
                                                  n * 512 + off + QW],
                                        start=(si == 0),
                                        stop=(si == len(segs) - 1))
                                if e == 0:
                                    # seed with the shared-expert partial
                                    nc.vector.scalar_tensor_tensor(
                                        pt[:, :], pY[:, :QW],
                                        cw[0][:, tt:tt + 1],
                                        ys[:, tt, n, off:off + QW],
                                        op0=ALU.mult, op1=ALU.add)
                                else:
                                    nc.vector.scalar_tensor_tensor(
                                        pt[:, :], pY[:, :QW],
                                        cw[1][:, tt:tt + 1],
                                        pt[:, :], op0=ALU.mult, op1=ALU.add)
                            if last_rep:
                                r0 = b * BLK + m * P
                                nc.scalar.dma_start(
                                    out=partial[q][r0:r0 + P, :],
                                    in_=pt[:, :])
                    if last_rep and with_collective:
                        for _cr in range(coll_reps):
                            nc.gpsimd.collective_compute(
                                "ReduceScatter", ALU.add,
                                replica_groups=[list(range(NCORES))],
                                ins=[partial[q][:]], outs=[rs[q][:]])

            # ------------- epilogue -------------
            for q in range(nq):
                if with_collective:
                    nc.sync.dma_start(out=out[:, q * QW:(q + 1) * QW],
                                      in_=rs[q][:])
                else:
                    nc.sync.dma_start(out=out[:, q * QW:(q + 1) * QW],
                                      in_=partial[q][0:P, :])

    _split_sync_waits(nc)
    return nc


def _perm_for_core(c):
    g_sel = c >> 1
    rot = 2 * (c & 1)
    perm = [4 * g_sel + ((rot + j) % 4) for j in range(4)]
    for g in range(4):
        if g != g_sel:
            perm.extend(range(4 * g, 4 * g + 4))
    return perm


def prepare_in_maps(h, gate_w, bias, wg, wu, wd, swg, swu, swd):
    bf = ml_dtypes.bfloat16
    h = np.asarray(h, np.float32)
    gate_w = np.asarray(gate_w, np.float32)
    bias = np.asarray(bias, np.float32)

    ht32 = np.ascontiguousarray(h.T)                      # [H, T] f32
    # blocked gate operand: [tt, p, k, t], 4KB contiguous per partition row
    ht32b = np.ascontiguousarray(
        ht32.reshape(KT, P, 8, P).transpose(2, 1, 0, 3))
    ht = ht32.astype(bf)                                  # [H, T] bf16
    gwt = np.ascontiguousarray(gate_w.T)                  # [H, E] f32

    swg32 = np.asarray(swg, np.float32)
    swu32 = np.asarray(swu, np.float32)
    swd32 = np.asarray(swd, np.float32)
    wg32 = np.asarray(wg, np.float32)
    wu32 = np.asarray(wu, np.float32)
    wd32 = np.asarray(wd, np.float32)

    def pack_cols(w, e0, e1):
        # columns: [e0 0:640 | e0 640:704 | e1 640:704 | e1 0:640]
        return np.concatenate([w[e0][:, :640], w[e0][:, 640:],
                               w[e1][:, 640:], w[e1][:, :640]], axis=1)

    in_maps = []
    for c in range(NCORES):
        e0, e1 = 2 * c, 2 * c + 1
        perm = _perm_for_core(c)
        csl = slice(c * SIC, (c + 1) * SIC)
        wdk = np.concatenate([wd32[e0][:640], wd32[e0][640:],
                              wd32[e1][640:], wd32[e1][:640]], axis=0)
        in_maps.append({
            "ht": ht,
            "ht32": ht32b,
            "gw32": np.ascontiguousarray(gwt[:, perm]),
            "bias_rep": np.tile(bias[perm], (P, 8)).astype(np.float32),
            "wgp": pack_cols(wg32, e0, e1).astype(bf),
            "wup": pack_cols(wu32, e0, e1).astype(bf),
            "wdk": np.ascontiguousarray(wdk).astype(bf),
            "swg_my": np.ascontiguousarray(swg32[:, csl]).astype(bf),
            "swu_my": np.ascontiguousarray(swu32[:, csl]).astype(bf),
            "swd_my": np.ascontiguousarray(swd32[csl, :]).astype(bf),
        })

    return in_maps


def get_nc(**kw):
    key = tuple(sorted(kw.items()))
    if key not in _BUILD_CACHE:
        _BUILD_CACHE[key] = _build(**kw)
    return _BUILD_CACHE[key]


def kernel(h, gate_w, bias, wg, wu, wd, swg, swu, swd):
    in_maps = prepare_in_maps(h, gate_w, bias, wg, wu, wd, swg, swu, swd)
    res = run_bass_kernel_spmd(get_nc(), in_maps, list(range(NCORES)))
    return np.concatenate([res.results[c]["out"] for c in range(NCORES)],
                          axis=0).astype(np.float32)

